# revision 1
# baseline (speedup 1.0000x reference)
"""GAT (2-layer, PPI config) on 8 trn2 NeuronCores.

Math: per layer, att = softmax_row(mask(leaky_relu(f_src[d] + f_dst[s]))).
With x = f_src + f_dst and alpha = 0.2:
    exp(lrelu(x)) = exp(x) * max(1, exp(-0.8 x))
                  = exp(f_src[d]) * exp(f_dst[s]) * G[s, d],
    G = max(1, R[d] * r[s]),  R = exp(-0.8 f_src), r = exp(-0.8 f_dst).
Softmax-normalizing cancels exp(f_src[d]); exp(f_dst[s]) folds into the
aggregation operand (Wh' = exp(f_dst) * Wh, plus a ones->exp(f_dst) column
that accumulates the softmax denominator).  Per (s, d) element the device
computes G then att = G * adj01, then a bf16 matmul.  Normalization/elu
happen on host.

Work is split between two engines per s-tile (measured per-[128,2048] op):
  DVE  : tensor_scalar G (4x mode, ~750ns), tensor_tensor mult mask
         (2x mode, ~1220ns)
  ACT  : G = Exp(Relu(-0.8 x)) as two LUT passes (~2000ns each)
n_act tiles per launch compute G on ACT.  For L1 all ACT G-ops sit on head
1 and write the odd half of a [128, 2*D] paired tile: ACT has no perf
modes to lose there, head 0's tensor_scalar keeps an offset-0 write (only
non-zero-offset DVE writes drop out of 4x mode), and ONE wide
tensor_tensor then masks both heads (~2290ns vs 2x ~1220ns) — the one
restructuring that reduced DVE work (L1 ~229 -> ~224us).  On plain tiles,
DVE-assigned heads issue first so their matmuls are not queued behind the
slower ACT pair (PE FIFO).  n_act=48 is a hard optimum: the HAM activity
throttle cliff sits immediately above (50 -> +12us, 52 -> +17us).  GpSimd
offload (n_gp) is OFF: its software mult plus Q7 semaphore traffic trips
the HAM activity throttle to a 0.5 utilization limit, stretching every
other engine ~20-40% (measured: 52% of the span at half rate vs 14%
without).  DMA issue order is tuned per layer: small G-inputs and the
first whp chunk ahead of the adjacency prefetch for L1 (compute-bound),
adjacency first for L2 (DMA-bound).

Sharding (8 cores), sized so each PSUM accumulator set fits (heads*D <= 4096
fp32 words per partition):
  L1 (4 heads): 4 destination ranges x 2 head-pairs, D=2048.
  L2 (1 head):  4 destination ranges x 2 source halves, D=2048; the host
                adds the two partial accumulator sets.
Two launches; the tiny inter-layer tensors are re-prepped on host.
"""

import os
import sys

sys.path.insert(0, "/opt/trn_rl_repo")

import numpy as np
import ml_dtypes

import concourse.bass as bass
import concourse.tile as tile
from concourse import bacc, mybir
from concourse.bass_utils import run_bass_kernel_spmd

BF16 = mybir.dt.bfloat16
F32 = mybir.dt.float32
FP8 = mybir.dt.float8e4
NPBF16 = ml_dtypes.bfloat16
NPFP8 = ml_dtypes.float8_e4m3

N = 8192
NFEAT = 256
NHID = 64
NHEADS = 4
NCLASS = 121
ALPHA = 0.2
N_CORES = 8
P = 128

_NC_CACHE = {}
_LAST_EXEC_NS = []


def _spread(n_sel, slots):
    """Evenly select n_sel items from the slot list (Bresenham)."""
    sel = set()
    if n_sel <= 0 or not slots:
        return sel
    nn = len(slots)
    for i, s in enumerate(slots):
        if (i * n_sel) % nn < n_sel:
            sel.add(s)
        if len(sel) == n_sel:
            break
    return sel


def build_att_kernel(n_heads, dh, n_stiles, D, warmup=20,
                     n_act=48, n_gp=14, adj_first=False, out_bf16=False,
                     n_conv=0, adj_bufs=6):
    """One attention-layer shard, per-core program.

    Inputs (per core):
      adjt [n_stiles*128, D]    bf16  adjacency slice (0/1), rows = source
                                      nodes, cols = destination range
      whp  [128, n_stiles*M]    bf16  pre-tiled stationary operand: per
                                      s-tile, per head, dh cols of
                                      exp(f_dst)*Wh then 1 col exp(f_dst)
      rsc  [128, n_stiles*H]    f32   pre-tiled r = exp(-0.8 f_dst)
      rbc  [128, H*D]           bf16  R = exp(-0.8 f_src[d_range]), bcast
      rbl  [128, H*D]           bf16  -0.8 f_src[d_range], bcast
      rsl  [128, n_stiles*H]    f32   -0.8 f_dst, pre-tiled
    Output:
      out [H*(dh+1), D] f32  raw accumulators: per head dh numerator rows
                             then 1 denominator row (normalize on host).
    """
    if n_heads == 2:
        # all ACT G-ops on head 1: its G lands in the odd half of a paired
        # tile (ACT has no perf modes to lose), head 0's ts keeps an
        # offset-0 write (4x), and one wide tt masks both heads.
        act_set = _spread(n_act, [(st, 1) for st in range(4, n_stiles)])
    else:
        act_set = _spread(n_act, [(st, h) for st in range(4, n_stiles)
                                  for h in range(n_heads)])
    gp_set = _spread(n_gp, list(range(2, n_stiles)))
    # s-tiles whose adjacency arrives as fp8 and is upcast on the scalar
    # engine (exact for 0/1 values); scheduled off the ACT G-tiles.
    conv_set = _spread(n_conv, [st for st in range(6, n_stiles)
                                if not any((st, h) in act_set
                                           for h in range(n_heads))])
    MP = 128  # stationary cols padded to 128 so FWL (fast weight load) engages
    M = n_heads * MP
    assert dh + 1 <= MP and n_heads * D * 4 <= 16384
    nc = bacc.Bacc("TRN2", target_bir_lowering=False, debug=False,
                   num_devices=N_CORES)
    adjt_d = nc.dram_tensor("adjt", [n_stiles * P, D], BF16,
                            kind="ExternalInput")
    adjt8_d = None
    if n_conv:
        adjt8_d = nc.dram_tensor("adjt8", [n_stiles * P, D], FP8,
                                 kind="ExternalInput")
    whp_d = nc.dram_tensor("whp", [P, n_stiles * M], BF16,
                           kind="ExternalInput")
    rsc_d = nc.dram_tensor("rsc", [P, n_stiles * n_heads], F32,
                           kind="ExternalInput")
    rbc_d = nc.dram_tensor("rbc", [P, n_heads * D], BF16,
                           kind="ExternalInput")
    rbl_d = nc.dram_tensor("rbl", [P, n_heads * D], BF16,
                           kind="ExternalInput")
    rsl_d = nc.dram_tensor("rsl", [P, n_stiles * n_heads], F32,
                           kind="ExternalInput")
    out_d = nc.dram_tensor("out", [n_heads * (dh + 1), D],
                           BF16 if out_bf16 else F32,
                           kind="ExternalOutput")

    with tile.TileContext(nc) as tc:
        with (
            tc.tile_pool(name="const", bufs=1) as cpool,
            tc.tile_pool(name="adj", bufs=adj_bufs) as apool,
            tc.tile_pool(name="g", bufs=4) as gpool,
            tc.tile_pool(name="att", bufs=4) as attpool,
            tc.tile_pool(name="g2", bufs=3) as g2pool,
            tc.tile_pool(name="att2", bufs=3) as att2pool,
            tc.tile_pool(name="fin", bufs=2) as fpool,
            tc.tile_pool(name="tmp", bufs=3) as tpool,
            tc.tile_pool(name="adj8", bufs=3) as a8pool,
            tc.tile_pool(name="acc", bufs=n_heads,
                         space=bass.MemorySpace.PSUM) as pspool,
        ):
            # DMA issue order matters: per-queue FIFOs serve the first
            # compute, so small G-inputs and the first whp chunk go first,
            # then the adjacency prefetch, then the bulk.
            adj_pre = []

            def load_adj_pre():
                for st in range(min(6, n_stiles)):
                    adjp = apool.tile([P, D], BF16, name=f"adjp{st}",
                                      tag="adj")
                    nc.sync.dma_start(adjp[:],
                                      adjt_d[st * P:(st + 1) * P, :])
                    adj_pre.append(adjp)

            def load_adj_pre2():
                for st in range(len(adj_pre), min(adj_bufs, n_stiles)):
                    adjp = apool.tile([P, D], BF16, name=f"adjp{st}",
                                      tag="adj")
                    nc.sync.dma_start(adjp[:],
                                      adjt_d[st * P:(st + 1) * P, :])
                    adj_pre.append(adjp)

            if adj_first:
                load_adj_pre()
            rsc = cpool.tile([P, n_stiles * n_heads], F32)
            nc.sync.dma_start(rsc[:], rsc_d[:])
            rsl = cpool.tile([P, n_stiles * n_heads], F32)
            nc.sync.dma_start(rsl[:], rsl_d[:])
            rbc = cpool.tile([P, n_heads * D], BF16)
            nc.sync.dma_start(rbc[:], rbc_d[:])
            wsplit = min(8, n_stiles) * M
            whp_a = cpool.tile([P, wsplit], BF16)
            nc.sync.dma_start(whp_a[:], whp_d[:, 0:wsplit])
            if not adj_first:
                load_adj_pre()
            load_adj_pre2()
            rbl = cpool.tile([P, n_heads * D], BF16)
            nc.sync.dma_start(rbl[:], rbl_d[:])
            whp_b = cpool.tile([P, n_stiles * M - wsplit], BF16)
            nc.sync.dma_start(whp_b[:], whp_d[:, wsplit:])

            accs = [pspool.tile([MP, D], F32, tag="acc", name=f"acc{i}")
                    for i in range(n_heads)]

            if warmup:
                # Dense matmul burst so the PE HAM un-throttles to 2.4 GHz
                # before the steady-state (sparser) matmul stream begins.
                wN = min(512, D)
                dmy = cpool.tile([P, wN], BF16)
                nc.vector.memset(dmy[:], 0.0)
                for w in range(warmup):
                    nc.tensor.matmul(accs[0][:, 0:wN],
                                     dmy[:, 0:wN][:, 0:MP] if wN >= MP
                                     else dmy[:, 0:wN],
                                     dmy[:, 0:wN], start=True, stop=True)

            for st in range(n_stiles):
                if st < len(adj_pre):
                    adj = adj_pre[st]
                elif st in conv_set:
                    adj8 = a8pool.tile([P, D], FP8, tag="adj8")
                    nc.sync.dma_start(adj8[:],
                                      adjt8_d[st * P:(st + 1) * P, :])
                    adj = apool.tile([P, D], BF16, tag="adj")
                    nc.scalar.activation(adj[:], adj8[:],
                                         mybir.ActivationFunctionType.Copy)
                else:
                    adj = apool.tile([P, D], BF16, tag="adj")
                    nc.sync.dma_start(adj[:], adjt_d[st * P:(st + 1) * P, :])
                if n_heads == 2 and (st, 1) in act_set \
                        and (st, 0) not in act_set:
                    # paired path: g2 = [G_h0 | G_h1], one wide mask mult
                    g2 = g2pool.tile([P, 2 * D], BF16, tag="g2p")
                    nc.vector.tensor_scalar(
                        g2[:, 0:D], rbc[:, 0:D],
                        rsc[:, st * n_heads:st * n_heads + 1],
                        1.0, mybir.AluOpType.mult, mybir.AluOpType.max)
                    t = tpool.tile([P, D], F32)
                    nc.scalar.activation(
                        t[:], rbl[:, D:2 * D],
                        mybir.ActivationFunctionType.Relu,
                        bias=rsl[:, st * n_heads + 1:st * n_heads + 2])
                    nc.scalar.activation(
                        g2[:, D:2 * D], t[:],
                        mybir.ActivationFunctionType.Exp)
                    att2 = att2pool.tile([P, 2 * D], BF16, tag="att2")
                    nc.vector.tensor_tensor(
                        att2[:].rearrange("p (r d) -> p r d", r=2),
                        g2[:].rearrange("p (r d) -> p r d", r=2),
                        adj[:].unsqueeze(1).broadcast_to([P, 2, D]),
                        mybir.AluOpType.mult)
                    for h in range(2):
                        w0 = st * M + h * MP
                        if w0 < wsplit:
                            lhs = whp_a[:, w0:w0 + MP]
                        else:
                            lhs = whp_b[:, w0 - wsplit:w0 - wsplit + MP]
                        for j0 in range(0, D, 512):
                            j1 = min(j0 + 512, D)
                            nc.tensor.matmul(
                                accs[h][:, j0:j1], lhs,
                                att2[:, h * D + j0:h * D + j1],
                                start=(st == 0), stop=(st == n_stiles - 1))
                    continue
                head_order = sorted(range(n_heads),
                                    key=lambda hh: (st, hh) in act_set)
                for h in head_order:
                    if (st, h) in act_set:
                        g = gpool.tile([P, D], BF16, tag="g", name="g_act")
                        t = tpool.tile([P, D], F32)
                        nc.scalar.activation(
                            t[:], rbl[:, h * D:(h + 1) * D],
                            mybir.ActivationFunctionType.Relu,
                            bias=rsl[:, st * n_heads + h:
                                     st * n_heads + h + 1])
                        nc.scalar.activation(
                            g[:], t[:], mybir.ActivationFunctionType.Exp)
                    else:
                        g = gpool.tile([P, D], BF16, tag="g")
                        nc.vector.tensor_scalar(
                            g[:], rbc[:, h * D:(h + 1) * D],
                            rsc[:, st * n_heads + h:st * n_heads + h + 1],
                            1.0, mybir.AluOpType.mult, mybir.AluOpType.max)
                    att = attpool.tile([P, D], BF16, tag="att")
                    nc.vector.tensor_tensor(att[:], g[:], adj[:],
                                            mybir.AluOpType.mult)
                    w0 = st * M + h * MP
                    if w0 < wsplit:
                        lhs = whp_a[:, w0:w0 + MP]
                    else:
                        lhs = whp_b[:, w0 - wsplit:w0 - wsplit + MP]
                    for j0 in range(0, D, 512):
                        j1 = min(j0 + 512, D)
                        nc.tensor.matmul(
                            accs[h][:, j0:j1], lhs, att[:, j0:j1],
                            start=(st == 0), stop=(st == n_stiles - 1))

            # Raw accumulators out; host normalizes (and applies elu).
            # Output DMA split into 32-row chunks to spread across queues.
            for h in range(n_heads):
                stg = fpool.tile([dh + 1, D], BF16 if out_bf16 else F32,
                                 tag="stg")
                if h % 2 == 0:
                    nc.vector.tensor_copy(stg[:], accs[h][0:dh + 1, :])
                else:
                    nc.scalar.copy(stg[:], accs[h][0:dh + 1, :])
                for c0 in range(0, dh + 1, 32):
                    c1 = min(c0 + 32, dh + 1)
                    nc.sync.dma_start(
                        out_d[h * (dh + 1) + c0:h * (dh + 1) + c1, :],
                        stg[c0:c1, :])

    nc.compile()
    return nc


def _get_kernel(n_heads, dh, n_stiles, D, n_act=48, n_gp=14,
                adj_first=False, out_bf16=False, n_conv=0, warmup=20,
                adj_bufs=6):
    key = (n_heads, dh, n_stiles, D, n_act, n_gp, adj_first, out_bf16,
           n_conv, warmup, adj_bufs)
    if key not in _NC_CACHE:
        _NC_CACHE[key] = build_att_kernel(n_heads, dh, n_stiles, D,
                                          warmup=warmup,
                                          n_act=n_act, n_gp=n_gp,
                                          adj_first=adj_first,
                                          out_bf16=out_bf16, n_conv=n_conv,
                                          adj_bufs=adj_bufs)
    return _NC_CACHE[key]


def _prep_core(Wh_heads, f_dst_heads, f_src_heads, dh, head_ids, s_range,
               d_range):
    """Host prep of whp / rsc / rbc / rbl / rsl for one core's shard."""
    s0, s1 = s_range
    n_st = (s1 - s0) // P
    H = len(head_ids)
    MP = 128
    M = H * MP
    Dc = d_range[1] - d_range[0]
    whp = np.zeros((P, n_st * M), dtype=NPBF16)
    rsc = np.empty((P, n_st * H), dtype=np.float32)
    rbc = np.empty((P, H * Dc), dtype=NPBF16)
    rbl = np.empty((P, H * Dc), dtype=NPBF16)
    rsl = np.empty((P, n_st * H), dtype=np.float32)
    for i, h in enumerate(head_ids):
        fd = f_dst_heads[h][s0:s1]
        v = np.exp(fd).astype(np.float32)
        r = np.exp(-(1.0 - ALPHA) * fd).astype(np.float32)
        whv = (Wh_heads[h][s0:s1] * v[:, None]).astype(np.float32)
        aug = np.concatenate([whv, v[:, None]], axis=1)  # [s1-s0, dh+1]
        tiled = aug.reshape(n_st, P, dh + 1).astype(NPBF16)
        for st in range(n_st):
            whp[:, st * M + i * MP:st * M + i * MP + dh + 1] = tiled[st]
        rsc[:, np.arange(n_st) * H + i] = r.reshape(n_st, P).T
        R = np.exp(-(1.0 - ALPHA)
                   * f_src_heads[h][d_range[0]:d_range[1]]).astype(NPBF16)
        rbc[:, i * Dc:(i + 1) * Dc] = R[None, :]
        rsl[:, np.arange(n_st) * H + i] = \
            (-(1.0 - ALPHA) * fd).astype(np.float32).reshape(n_st, P).T
        rbl[:, i * Dc:(i + 1) * Dc] = (-(1.0 - ALPHA)
            * f_src_heads[h][d_range[0]:d_range[1]]).astype(NPBF16)[None, :]
    return whp, rsc, rbc, rbl, rsl


def _launch(nc, in_maps):
    trace = bool(os.environ.get("GAT_TRACE"))
    res = run_bass_kernel_spmd(nc, in_maps, list(range(N_CORES)), trace=trace)
    if trace:
        _LAST_EXEC_NS.append(res.exec_time_ns)
    return [res.results[c]["out"] for c in range(N_CORES)]


def kernel(x, adj, Ws, a_heads, W_out, a_out):
    _LAST_EXEC_NS.clear()
    x = np.asarray(x, dtype=np.float32)
    adj = np.asarray(adj, dtype=np.float32)
    Ws = np.asarray(Ws, dtype=np.float32)
    a_heads = np.asarray(a_heads, dtype=np.float32)
    W_out = np.asarray(W_out, dtype=np.float32)
    a_out = np.asarray(a_out, dtype=np.float32)

    # ---- Layer 1: 4 d-ranges (D=2048) x 2 head-pairs ----
    D1 = N // 4
    Wh = [x @ Ws[h] for h in range(NHEADS)]
    f_src = [Wh[h] @ a_heads[h][:NHID] for h in range(NHEADS)]
    f_dst = [Wh[h] @ a_heads[h][NHID:] for h in range(NHEADS)]
    nc1 = _get_kernel(2, NHID, N // P, D1, n_act=48, n_gp=0)
    adjt_q = [np.ascontiguousarray(adj[q * D1:(q + 1) * D1, :].T
                                   .astype(NPBF16))
              for q in range(4)]
    in_maps = []
    for c in range(N_CORES):
        hg, q = c // 4, c % 4
        whp, rsc, rbc, rbl, rsl = _prep_core(Wh, f_dst, f_src, NHID,
                                             [2 * hg, 2 * hg + 1], (0, N),
                                             (q * D1, (q + 1) * D1))
        in_maps.append({"adjt": adjt_q[q], "whp": whp, "rsc": rsc,
                        "rbc": rbc, "rbl": rbl, "rsl": rsl})
    outs = _launch(nc1, in_maps)
    h_cat = np.empty((N, NHEADS * NHID), dtype=np.float32)
    for c in range(N_CORES):
        hg, q = c // 4, c % 4
        o = outs[c]  # [2*(NHID+1), D1]
        for i in range(2):
            h = 2 * hg + i
            num = o[i * (NHID + 1):i * (NHID + 1) + NHID, :]
            den = o[i * (NHID + 1) + NHID, :]
            ht = (num / den[None, :]).T  # [D1, NHID]
            h_cat[q * D1:(q + 1) * D1, h * NHID:(h + 1) * NHID] = \
                np.where(ht > 0, ht, np.expm1(np.minimum(ht, 0)))

    # ---- Layer 2: 4 d-ranges (D=2048) x 2 source halves ----
    Wh2 = h_cat @ W_out
    f_src2 = Wh2 @ a_out[:NCLASS]
    f_dst2 = Wh2 @ a_out[NCLASS:]
    nc2 = _get_kernel(1, NCLASS, N // 2 // P, D1, n_act=13, n_gp=0,
                      adj_first=True)
    in_maps = []
    for c in range(N_CORES):
        sh, q = c // 4, c % 4
        s_range = (sh * (N // 2), (sh + 1) * (N // 2))
        whp, rsc, rbc, rbl, rsl = _prep_core([Wh2], [f_dst2], [f_src2],
                                             NCLASS, [0], s_range,
                                             (q * D1, (q + 1) * D1))
        adjt = np.ascontiguousarray(
            adj[q * D1:(q + 1) * D1, s_range[0]:s_range[1]].T.astype(NPBF16))
        in_maps.append({"adjt": adjt, "whp": whp, "rsc": rsc, "rbc": rbc,
                        "rbl": rbl, "rsl": rsl})
    outs2 = _launch(nc2, in_maps)
    out = np.empty((N, NCLASS), dtype=np.float32)
    for q in range(4):
        o = outs2[q] + outs2[q + 4]  # add the two source-half partials
        out[q * D1:(q + 1) * D1, :] = (o[:NCLASS, :]
                                       / o[NCLASS, :][None, :]).T
    return out



# revision 2
# speedup vs baseline: 1.3522x; 1.3522x over previous
"""GAT (2-layer, PPI config) on 8 trn2 NeuronCores — sorted-tile scheme.

Math: att_unnorm[d,s] = exp(lrelu(f_src[d]+f_dst[s])) * adj[d,s].  With
x = f_src[d]+f_dst[s]:
    exp(lrelu(x)) = exp(x) * max(1, exp(-0.8x))
and after dropping the row-constant exp(f_src[d]) (softmax cancels it):
    w[d,s] = exp(f_dst[s]) * max(1, R[d] r[s]) * adj,
    R = exp(-0.8 f_src), r = exp(-0.8 f_dst).
Key identity: on any tile where x >= 0 everywhere, w = exp(f_dst)*adj, so
the matmul rhs is the RAW adjacency tile (lhs = whp = exp(f_dst)*[Wh|1]).
Where x < 0 everywhere, w = exp(0.2 f_dst) * R[d] * adj: rhs is again raw
adjacency with lhs = whpr = exp(0.2 f_dst)*[Wh|1], and the per-column
R[d] scale is applied by the HOST on the dumped accumulator.  Only tiles
straddling x = 0 need explicit per-element att (ACT fp8->bf16 upcast +
DVE tensor_scalar G + tensor_tensor mask).

Sorting source rows by f_dst and destination cols by f_src (host-side
permutation, free) makes sign-pure tiles the overwhelming majority: the
s-axis splits per 512-col chunk into a neg-prefix [0,k_c), an explicit
window [k_c,p_c), and a pos-suffix [p_c,n_st).  One SPMD program serves
all 8 cores, so (k_c, p_c) are the min/max over the per-core exact
bounds (window ~15% of tiles; correctness never depends on the split —
the explicit path is exact everywhere).

adj ships as fp8e4 (exact for 0/1), halving HBM traffic; matmuls run
mixed bf16 lhs x fp8 rhs.  PSUM holds one [128, D] f32 accumulator; each
512-col chunk = one PSUM bank runs two sequential accumulation groups
(neg then window+pos) with a mid-stream dump of the neg partial.

Sharding:
  L1: 8 cores = 4 heads x 2 column-halves (interleaved 512-blocks of the
      per-head f_src-descending order, so chunk quantiles align across
      cores).  Per core: all 8192 sources (64 tiles, f_dst-ascending),
      D=4096.
  L2: 8 cores = 4 column-quarters x 2 source-halves (d-blocks {4c+q},
      s-tiles {2t+sh} interleaved).  Per core: 32 s-tiles, D=2048.
Host: normalize num/den, apply R on neg partials, elu, un-permute.
"""

import os
import sys

sys.path.insert(0, "/opt/trn_rl_repo")

import numpy as np
import ml_dtypes

import concourse.bass as bass
import concourse.tile as tile
from concourse import bacc, mybir
from concourse.bass_utils import run_bass_kernel_spmd

BF16 = mybir.dt.bfloat16
F32 = mybir.dt.float32
FP8 = mybir.dt.float8e4
NPBF16 = ml_dtypes.bfloat16
NPFP8 = ml_dtypes.float8_e4m3

N = 8192
NFEAT = 256
NHID = 64
NHEADS = 4
NCLASS = 121
ALPHA = 0.2
N_CORES = 8
P = 128
CH = 512  # chunk width = one PSUM bank of f32

_NC_CACHE = {}
_LAST_EXEC_NS = []
_WINDOW_STATS = []


def build_sorted_kernel(n_st, D, dh, kcs, pcs, warmup=20, adj_pre=12,
                        adj_bufs=14, wsplit_t=8):
    """One attention layer shard, shared SPMD program.

    Inputs (per core):
      adjt8 [n_st*128, D] fp8   adjacency slice, rows = sorted sources,
                                cols = per-core sorted dest blocks
      whp   [128, n_st*128] bf16  pos-phase lhs: exp(f_dst)*[Wh|1], padded
      whpr  [128, n_st*128] bf16  neg-phase lhs: exp(0.2 f_dst)*[Wh|1]
      rbc   [128, D]       bf16  R = exp(-0.8 f_src[cols]), row-broadcast
      rsc   [128, n_st]    f32   r = exp(-0.8 f_dst), per s-tile column
    Output:
      out [2*(dh+1), D] f32  rows 0:dh+1 = neg partial (host scales by R),
                             rows dh+1:  = window+pos partial.
    """
    NCH = D // CH
    assert len(kcs) == NCH and len(pcs) == NCH
    MP = 128
    nc = bacc.Bacc("TRN2", target_bir_lowering=False, debug=False,
                   num_devices=N_CORES)
    adjt_d = nc.dram_tensor("adjt8", [n_st * P, D], FP8, kind="ExternalInput")
    whp_d = nc.dram_tensor("whp", [P, n_st * MP], BF16, kind="ExternalInput")
    whpr_d = nc.dram_tensor("whpr", [P, n_st * MP], BF16,
                            kind="ExternalInput")
    rbc_d = nc.dram_tensor("rbc", [P, D], BF16, kind="ExternalInput")
    rsc_d = nc.dram_tensor("rsc", [P, n_st], F32, kind="ExternalInput")
    out_d = nc.dram_tensor("out", [2 * (dh + 1), D], F32,
                           kind="ExternalOutput")

    with tile.TileContext(nc) as tc:
        with (
            tc.tile_pool(name="const", bufs=1) as cpool,
            tc.tile_pool(name="adj", bufs=adj_bufs) as apool,
            tc.tile_pool(name="adjb", bufs=4) as bpool,
            tc.tile_pool(name="g", bufs=4) as gpool,
            tc.tile_pool(name="att", bufs=4) as attpool,
            tc.tile_pool(name="acc", bufs=1,
                         space=bass.MemorySpace.PSUM) as pspool,
        ):
            # Small tensors and the first lhs tiles first (per-queue FIFOs
            # serve the first compute), then the adjacency stream.
            rsc = cpool.tile([P, n_st], F32)
            nc.sync.dma_start(rsc[:], rsc_d[:])
            rbc = cpool.tile([P, D], BF16)
            nc.sync.dma_start(rbc[:], rbc_d[:])
            wsplit = min(wsplit_t, n_st) * MP
            whp_a = cpool.tile([P, wsplit], BF16)
            nc.sync.dma_start(whp_a[:], whp_d[:, 0:wsplit])
            whpr_a = cpool.tile([P, wsplit], BF16)
            nc.sync.dma_start(whpr_a[:], whpr_d[:, 0:wsplit])

            adj_tiles = []

            def issue_adj(st):
                adjp = apool.tile([P, D], FP8, name=f"adj{st}", tag="adj")
                nc.sync.dma_start(adjp[:], adjt_d[st * P:(st + 1) * P, :])
                adj_tiles.append(adjp)

            for st in range(min(adj_pre, n_st)):
                issue_adj(st)

            whp_b = None
            whpr_b = None
            if n_st * MP > wsplit:
                whp_b = cpool.tile([P, n_st * MP - wsplit], BF16)
                nc.sync.dma_start(whp_b[:], whp_d[:, wsplit:])
                whpr_b = cpool.tile([P, n_st * MP - wsplit], BF16)
                nc.sync.dma_start(whpr_b[:], whpr_d[:, wsplit:])

            def lhs_slice(buf_a, buf_b, st):
                w0 = st * MP
                if w0 < wsplit:
                    return buf_a[:, w0:w0 + MP]
                return buf_b[:, w0 - wsplit:w0 - wsplit + MP]

            acc = pspool.tile([MP, D], F32, tag="acc")
            negstage = cpool.tile([dh + 1, D], F32)
            posstage = cpool.tile([dh + 1, D], F32)

            if warmup:
                # Dense matmul burst so the PE HAM un-throttles to 2.4 GHz
                # before the steady-state matmul stream begins.
                wN = min(512, D)
                dmy = cpool.tile([P, wN], BF16)
                nc.vector.memset(dmy[:], 0.0)
                for w in range(warmup):
                    nc.tensor.matmul(acc[:, 0:wN], dmy[:, 0:MP],
                                     dmy[:, 0:wN], start=True, stop=True)

            for st in range(n_st):
                if st + adj_pre < n_st:
                    issue_adj(st + adj_pre)
                adj = adj_tiles[st]
                # explicit-att window tiles: ACT upcast + DVE ts/tt
                atts = {}
                for c in range(NCH):
                    if kcs[c] <= st < pcs[c]:
                        sl = slice(c * CH, (c + 1) * CH)
                        adjb = bpool.tile([P, CH], BF16, tag="adjb")
                        nc.scalar.activation(
                            adjb[:], adj[:, sl],
                            mybir.ActivationFunctionType.Copy)
                        g = gpool.tile([P, CH], BF16, tag="g")
                        nc.vector.tensor_scalar(
                            g[:], rbc[:, sl], rsc[:, st:st + 1], 1.0,
                            mybir.AluOpType.mult, mybir.AluOpType.max)
                        att = attpool.tile([P, CH], BF16, tag="att")
                        nc.vector.tensor_tensor(att[:], g[:], adjb[:],
                                                mybir.AluOpType.mult)
                        atts[c] = att
                # matmuls: pos/window chunks (prefix, lhs=whp) then neg
                # chunks (suffix, lhs=whpr) — two weight loads per s-tile
                for c in range(NCH):
                    sl = slice(c * CH, (c + 1) * CH)
                    if st < kcs[c]:
                        continue
                    if c in atts:
                        rhs = atts[c][:]
                    else:
                        rhs = adj[:, sl]
                    nc.tensor.matmul(acc[:, sl],
                                     lhs_slice(whp_a, whp_b, st), rhs,
                                     start=(st == kcs[c]),
                                     stop=(st == n_st - 1))
                for c in range(NCH):
                    sl = slice(c * CH, (c + 1) * CH)
                    if st < kcs[c]:
                        nc.tensor.matmul(acc[:, sl],
                                         lhs_slice(whpr_a, whpr_b, st),
                                         adj[:, sl],
                                         start=(st == 0),
                                         stop=(st == kcs[c] - 1))
                # neg-partial dumps as each chunk's neg group closes
                for c in range(NCH):
                    if 0 < kcs[c] and st == kcs[c] - 1:
                        sl = slice(c * CH, (c + 1) * CH)
                        if c % 2 == 0:
                            nc.vector.tensor_copy(negstage[:, sl],
                                                  acc[0:dh + 1, sl])
                        else:
                            nc.scalar.copy(negstage[:, sl],
                                           acc[0:dh + 1, sl])
                        for r0 in range(0, dh + 1, 32):
                            r1 = min(r0 + 32, dh + 1)
                            nc.sync.dma_start(out_d[r0:r1, sl],
                                              negstage[r0:r1, sl])

            for c in range(NCH):
                sl = slice(c * CH, (c + 1) * CH)
                if kcs[c] < n_st:
                    if c % 2 == 0:
                        nc.vector.tensor_copy(posstage[:, sl],
                                              acc[0:dh + 1, sl])
                    else:
                        nc.scalar.copy(posstage[:, sl], acc[0:dh + 1, sl])
                    for r0 in range(0, dh + 1, 32):
                        r1 = min(r0 + 32, dh + 1)
                        nc.sync.dma_start(
                            out_d[dh + 1 + r0:dh + 1 + r1, sl],
                            posstage[r0:r1, sl])

    nc.compile()
    return nc


def _get_kernel(n_st, D, dh, kcs, pcs, **kw):
    key = (n_st, D, dh, tuple(kcs), tuple(pcs), tuple(sorted(kw.items())))
    if key not in _NC_CACHE:
        _NC_CACHE[key] = build_sorted_kernel(n_st, D, dh, list(kcs),
                                             list(pcs), **kw)
    return _NC_CACHE[key]


def _classify(f_src_cols, tmin, tmax, n_st):
    """Per 512-col chunk: (n, p) = end of all-neg prefix / start of
    all-pos suffix, given sorted s-tile f_dst mins/maxes."""
    res = []
    for c0 in range(0, len(f_src_cols), CH):
        fs = f_src_cols[c0:c0 + CH]
        T1, T2 = -fs.max(), -fs.min()
        nn = int((tmax < T1).sum())
        p_arr = np.nonzero(tmin >= T2)[0]
        pp = int(p_arr[0]) if len(p_arr) else n_st
        res.append((nn, max(pp, nn)))
    return res


def _prep_lhs(Wh_s, f_dst_s, dh, n_st):
    """whp / whpr stationary buffers from sorted-row Wh and f_dst."""
    MP = 128
    v = np.exp(f_dst_s).astype(np.float32)
    vr = np.exp(ALPHA * f_dst_s).astype(np.float32)
    aug = np.concatenate([Wh_s, np.ones((len(f_dst_s), 1), np.float32)],
                         axis=1)  # [S, dh+1]
    whp = np.zeros((P, n_st * MP), dtype=NPBF16)
    whpr = np.zeros((P, n_st * MP), dtype=NPBF16)
    a1 = (aug * v[:, None]).astype(NPBF16).reshape(n_st, P, dh + 1)
    a2 = (aug * vr[:, None]).astype(NPBF16).reshape(n_st, P, dh + 1)
    for st in range(n_st):
        whp[:, st * MP:st * MP + dh + 1] = a1[st]
        whpr[:, st * MP:st * MP + dh + 1] = a2[st]
    return whp, whpr


def _launch(nc, in_maps):
    trace = bool(os.environ.get("GAT_TRACE"))
    res = run_bass_kernel_spmd(nc, in_maps, list(range(N_CORES)),
                               trace=trace)
    if trace:
        _LAST_EXEC_NS.append(res.exec_time_ns)
    return [res.results[c]["out"] for c in range(N_CORES)]


def _run_layer(adjT8, Wh_heads, f_src_heads, f_dst_heads, dh, core_specs,
               n_st, D, **kw):
    """core_specs: list of (head, d_cols, s_rows_sorted_idx) per core.
    Returns per-core (neg, pos) accumulators plus shared kcs/pcs."""
    n_cores = len(core_specs)
    cls = []
    for (h, d_cols, s_idx) in core_specs:
        fd = f_dst_heads[h][s_idx]
        tmin = fd.reshape(n_st, P).min(1)
        tmax = fd.reshape(n_st, P).max(1)
        cls.append(_classify(f_src_heads[h][d_cols], tmin, tmax, n_st))
    cls = np.array(cls)  # [cores, NCH, 2]
    kcs = cls[:, :, 0].min(0)
    pcs = cls[:, :, 1].max(0)
    _WINDOW_STATS.append(int((pcs - kcs).sum()))
    nc = _get_kernel(n_st, D, dh, kcs, pcs, **kw)
    in_maps = []
    for (h, d_cols, s_idx) in core_specs:
        whp, whpr = _prep_lhs(Wh_heads[h][s_idx], f_dst_heads[h][s_idx],
                              dh, n_st)
        rbc = np.broadcast_to(
            np.exp(-(1 - ALPHA) * f_src_heads[h][d_cols]).astype(NPBF16),
            (P, D)).copy()
        rsc = np.ascontiguousarray(
            np.exp(-(1 - ALPHA) * f_dst_heads[h][s_idx])
            .astype(np.float32).reshape(n_st, P).T)
        adjt8 = adjT8[np.ix_(s_idx, d_cols)]
        in_maps.append({"adjt8": np.ascontiguousarray(adjt8), "whp": whp,
                        "whpr": whpr, "rbc": rbc, "rsc": rsc})
    outs = _launch(nc, in_maps)
    return outs, kcs, pcs


def _combine(out, kcs, pcs, f_src_cols, dh, n_st):
    """raw = pos + R*neg per chunk, honoring which groups exist."""
    neg = out[0:dh + 1, :].astype(np.float32)
    pos = out[dh + 1:, :].astype(np.float32)
    raw = np.zeros_like(pos)
    R = np.exp(-(1 - ALPHA) * f_src_cols).astype(np.float32)
    for c in range(len(kcs)):
        sl = slice(c * CH, (c + 1) * CH)
        if kcs[c] >= n_st:
            raw[:, sl] = R[sl][None, :] * neg[:, sl]
        elif kcs[c] > 0:
            raw[:, sl] = pos[:, sl] + R[sl][None, :] * neg[:, sl]
        else:
            raw[:, sl] = pos[:, sl]
    return raw


def kernel(x, adj, Ws, a_heads, W_out, a_out):
    _LAST_EXEC_NS.clear()
    _WINDOW_STATS.clear()
    x = np.asarray(x, dtype=np.float32)
    adj = np.asarray(adj, dtype=np.float32)
    Ws = np.asarray(Ws, dtype=np.float32)
    a_heads = np.asarray(a_heads, dtype=np.float32)
    W_out = np.asarray(W_out, dtype=np.float32)
    a_out = np.asarray(a_out, dtype=np.float32)

    # adjT8[s, d] = adj[d, s] as fp8 (exact 0/1), shared by both layers
    adjT8 = adj.T.astype(NPFP8)

    # ---- Layer 1: 4 heads x 2 col-halves, full s ----
    Wh = [x @ Ws[h] for h in range(NHEADS)]
    f_src = [Wh[h] @ a_heads[h][:NHID] for h in range(NHEADS)]
    f_dst = [Wh[h] @ a_heads[h][NHID:] for h in range(NHEADS)]
    d_orders = [np.argsort(-f_src[h]) for h in range(NHEADS)]
    s_orders = [np.argsort(f_dst[h]) for h in range(NHEADS)]
    D1, n_st1 = N // 2, N // P
    core_specs = []
    for c in range(N_CORES):
        h, j = c // 2, c % 2
        blocks = [d_orders[h][(2 * cc + j) * CH:(2 * cc + j + 1) * CH]
                  for cc in range(D1 // CH)]
        core_specs.append((h, np.concatenate(blocks), s_orders[h]))
    outs, kcs, pcs = _run_layer(adjT8, Wh, f_src, f_dst, NHID, core_specs,
                                n_st1, D1)
    h_cat = np.empty((N, NHEADS * NHID), dtype=np.float32)
    for c in range(N_CORES):
        h, d_cols = core_specs[c][0], core_specs[c][1]
        raw = _combine(outs[c], kcs, pcs, f_src[h][d_cols], NHID, n_st1)
        ht = (raw[0:NHID, :] / raw[NHID, :][None, :]).T  # [D1, NHID]
        ht = np.where(ht > 0, ht, np.expm1(np.minimum(ht, 0)))
        h_cat[d_cols, h * NHID:(h + 1) * NHID] = ht

    # ---- Layer 2: 4 col-quarters x 2 s-halves, 1 head ----
    Wh2 = h_cat @ W_out
    f_src2 = Wh2 @ a_out[:NCLASS]
    f_dst2 = Wh2 @ a_out[NCLASS:]
    d_order2 = np.argsort(-f_src2)
    s_order2 = np.argsort(f_dst2)
    D2, n_st2 = N // 4, N // 2 // P
    core_specs2 = []
    for c in range(N_CORES):
        q, sh = c % 4, c // 4
        blocks = [d_order2[(4 * cc + q) * CH:(4 * cc + q + 1) * CH]
                  for cc in range(D2 // CH)]
        s_idx = np.concatenate(
            [s_order2[(2 * t + sh) * P:(2 * t + sh + 1) * P]
             for t in range(n_st2)])
        core_specs2.append((0, np.concatenate(blocks), s_idx))
    outs2, kcs2, pcs2 = _run_layer(adjT8, [Wh2], [f_src2], [f_dst2],
                                   NCLASS, core_specs2, n_st2, D2)
    out = np.empty((N, NCLASS), dtype=np.float32)
    for q in range(4):
        d_cols = core_specs2[q][1]
        raw = (_combine(outs2[q], kcs2, pcs2, f_src2[d_cols], NCLASS,
                        n_st2)
               + _combine(outs2[q + 4], kcs2, pcs2, f_src2[d_cols],
                          NCLASS, n_st2))
        out[d_cols, :] = (raw[0:NCLASS, :] / raw[NCLASS, :][None, :]).T
    return out
